# revision 20
# baseline (speedup 1.0000x reference)
"""CrossLingualAttention Trainium2 kernel.

Sharding: data-parallel over batch B=8 across 8 NeuronCores, one batch
element per core; all parameters replicated (gathered per-core on host
by language id). One SPMD Bass/Tile program; per-core input maps carry
the core's batch slice and its language-specialized weights.

Math (per batch element i, lang l = language_ids[i]):
  q = h @ Wq[l].T + bq[l];   Q = q @ wq.T + bq_      -> folded on host:
      Q = h @ WcqT + bcq,    WcqT = (wq @ Wq[l]).T,  bcq = wq@bq[l]+bq_
  (same for K);  V = h @ wv.T + bv
  attn = softmax(Q K^T / 8) per head (no masking -- reference's
      key_padding_mask bug makes the mask a no-op)
  ctx = attn @ V
  post-attention:  out = (ctx @ Wo^T + bo) @ S1..S7 @ Wp^T + bp
      = ctx @ G + b_post
      G = Wo^T @ S1..S7 @ Wp^T   -- computed on device as a balanced
          product tree (critical path 4 matmul levels, width 4) with
          leaves pre-oriented on host so every node is one matmul pass
      b_post = bo @ S1..S7 @ Wp^T + bp   (vector chain, host)
  x = out + h;  layernorm(x) * g + b

On-device layout: activations transposed [H(part), S(free)] so every
linear is a plain lhsT.T@rhs matmul; attention uses scoresT [k, q]
layout (softmax denominator via a fused ones-column in the V operand),
deferring normalization to a reciprocal-broadcast multiply; final PE
transpose feeds the layernorm in natural [S, H] layout. Matmul operands
are bf16 (fp32 PSUM accumulation); residual/layernorm stay fp32.
"""

import ml_dtypes
import numpy as np
from contextlib import ExitStack

import concourse.bacc as bacc
import concourse.mybir as mybir
import concourse.tile as tile
from concourse.bass_utils import run_bass_kernel_spmd
from concourse.masks import make_identity

BF = ml_dtypes.bfloat16
F32 = mybir.dt.float32
BF16 = mybir.dt.bfloat16
AF = mybir.ActivationFunctionType
ALU = mybir.AluOpType

B, S, H, NH, NL = 8, 768, 768, 12, 5
HD = H // NH          # 64
P = 128
NT = H // P           # 6 partition tiles
FC = 384              # free-dim chunk (<=512 fp32 psum bank)
NFC = S // FC         # 2
NCHAIN = 7            # alignment factors per batch element (j != i)

_CACHED_NC = None
LAST_RESULTS = None


def _build_program():
    nc = bacc.Bacc(None)

    # ---- per-core DRAM inputs ----
    d_ht = nc.dram_tensor("ht", [H, S], F32, kind="ExternalInput")
    d_htb = nc.dram_tensor("htb", [H, S], BF16, kind="ExternalInput")
    d_wcq = nc.dram_tensor("wcq", [H, H], BF16, kind="ExternalInput")  # [in, out]
    d_wck = nc.dram_tensor("wck", [H, H], BF16, kind="ExternalInput")
    d_wv = nc.dram_tensor("wv", [H, H], BF16, kind="ExternalInput")    # wv.T
    d_r0 = nc.dram_tensor("r0", [H, H], BF16, kind="ExternalInput")    # out_proj_w
    d_chain = nc.dram_tensor("chain", [NCHAIN, H, H], BF16, kind="ExternalInput")
    d_projT = nc.dram_tensor("projT", [H, H], BF16, kind="ExternalInput")
    d_bias = nc.dram_tensor("bias_pack", [NT, P, 5], F32, kind="ExternalInput")
    d_rows = nc.dram_tensor("rows_pack", [1, S], F32, kind="ExternalInput")
    d_out = nc.dram_tensor("out", [H, S], F32, kind="ExternalOutput")
    import os as _os
    _dbg = _os.environ.get("KDEBUG") == "1"
    if _dbg:
        d_dbg = nc.dram_tensor("dbg", [NT, P, S], F32, kind="ExternalOutput")
        d_dbgrow = nc.dram_tensor("dbgrow", [3, S], F32, kind="ExternalOutput")

    ht_t = d_ht.rearrange("(t p) s -> t p s", p=P)
    htb_t = d_htb.rearrange("(t p) s -> t p s", p=P)
    wcq_t = d_wcq.rearrange("(t p) o -> t p o", p=P)
    wck_t = d_wck.rearrange("(t p) o -> t p o", p=P)
    wv_t = d_wv.rearrange("(t p) o -> t p o", p=P)
    r0_t = d_r0.rearrange("(t p) o -> t p o", p=P)
    chain_t = d_chain.rearrange("c (t p) o -> c t p o", p=P)
    projT_t = d_projT.rearrange("(t p) o -> t p o", p=P)
    out_t = d_out.rearrange("(t p) s -> t p s", p=P)

    with ExitStack() as ctx:
        tc = ctx.enter_context(tile.TileContext(nc))
        # SBUF pools
        per = ctx.enter_context(tc.tile_pool(name="persist", bufs=1))
        wp = ctx.enter_context(tc.tile_pool(name="wpool", bufs=14))
        rp = ctx.enter_context(tc.tile_pool(name="rpool", bufs=30))
        ap_ = ctx.enter_context(tc.tile_pool(name="apool", bufs=8))
        sm = ctx.enter_context(tc.tile_pool(name="small", bufs=1))
        bc = ctx.enter_context(tc.tile_pool(name="bcast", bufs=2))
        rbp = ctx.enter_context(tc.tile_pool(name="rbp", bufs=3))
        # PSUM pools: pS holds [128, 2x512-padded] (2 banks) x2; pA 3; pC 1
        pA = ctx.enter_context(tc.tile_pool(name="pA", bufs=2, space="PSUM"))
        pS = ctx.enter_context(tc.tile_pool(name="pS", bufs=2, space="PSUM"))
        pC = ctx.enter_context(tc.tile_pool(name="pC", bufs=2, space="PSUM"))

        # ---- small constants ----
        identity = sm.tile([P, P], F32)
        make_identity(nc, identity)
        ones1 = sm.tile([1, P], F32)
        nc.vector.memset(ones1, 1.0)
        ones64 = sm.tile([1, HD], F32)
        nc.vector.memset(ones64, 1.0)
        eps_t = sm.tile([P, 1], F32)
        nc.vector.memset(eps_t, 1e-5)

        # ---- load hidden (bf16 matmul operand) first ----
        htb = []
        for t in range(NT):
            hb = per.tile([P, S], BF16, name=f"htb{t}", tag=f"htb{t}")
            [nc.sync, nc.scalar][t % 2].dma_start(out=hb, in_=htb_t[t])
            htb.append(hb)

        bias_sb = sm.tile([P, NT, 5], F32)
        for t in range(NT):
            nc.gpsimd.dma_start(out=bias_sb[:, t, :], in_=d_bias[t])
        row_bv = sm.tile([1, S], F32)
        nc.gpsimd.dma_start(out=row_bv, in_=d_rows[0:1])

        dma_engs = [nc.sync, nc.scalar]

        def load_w(dram_tiled, nm):
            tiles = []
            for t in range(NT):
                w = wp.tile([P, H], BF16, name=f"{nm}{t}", tag="w")
                dma_engs[t % 2].dma_start(out=w, in_=dram_tiled[t])
                tiles.append(w)
            return tiles

        def linear_T(w_tiles, x_tiles, out_tiles, bias_col):
            """out[o,s] tiles = sum_i w[i,o]^T x[i,s] + bias (per-partition)."""
            for m in range(NT):
                for c in range(NFC):
                    ps = pA.tile([P, FC], F32, name="ps_lin", tag="ps_lin")
                    for k in range(NT):
                        nc.tensor.matmul(
                            ps, lhsT=w_tiles[k][:, m * P:(m + 1) * P],
                            rhs=x_tiles[k][:, c * FC:(c + 1) * FC],
                            start=(k == 0), stop=(k == NT - 1))
                    nc.vector.tensor_scalar_add(
                        out=out_tiles[m][:, c * FC:(c + 1) * FC],
                        in0=ps, scalar1=bias_sb[:, m, bias_col:bias_col + 1])

        # ---- phase A/B: Q^T, K^T ----
        wq_tiles = load_w(wcq_t, "wq")
        QT = [per.tile([P, S], BF16, name=f"QT{t}", tag=f"QT{t}") for t in range(NT)]
        linear_T(wq_tiles, htb, QT, 0)

        wk_tiles = load_w(wck_t, "wk")
        KT = [per.tile([P, S], BF16, name=f"KT{t}", tag=f"KT{t}") for t in range(NT)]
        linear_T(wk_tiles, htb, KT, 1)

        # ---- phase C: V in natural layout [s, (head, 65)] with ones column ----
        # broadcast bv row -> [128, S] (K=1 ones matmul)
        bvb = bc.tile([P, S], F32, name="bvb", tag="bcast")
        for c in range(NFC):
            ps = pA.tile([P, FC], F32, name="ps_bc", tag="ps_lin")
            nc.tensor.matmul(ps, lhsT=ones1, rhs=row_bv[:, c * FC:(c + 1) * FC],
                             start=True, stop=True)
            nc.vector.tensor_copy(out=bvb[:, c * FC:(c + 1) * FC], in_=ps)
        # residual copy of hidden (fp32), needed from the apply phase on
        ht = []
        for t in range(NT):
            h_tile = per.tile([P, S], F32, name=f"ht{t}", tag=f"ht{t}")
            nc.gpsimd.dma_start(out=h_tile, in_=ht_t[t])
            ht.append(h_tile)
        wv_tiles = load_w(wv_t, "wv")
        Vsb = [per.tile([P, NH, HD + 1], BF16, name=f"V{t}", tag=f"V{t}")
               for t in range(NT)]
        for t in range(NT):
            nc.vector.memset(Vsb[t][:, :, HD:HD + 1], 1.0)
        for m in range(NT):      # m: s-tile
            for c in range(NFC):  # c: 384-wide chunk = 6 heads
                ps = pA.tile([P, FC], F32, name="ps_v", tag="ps_lin")
                for k in range(NT):
                    nc.tensor.matmul(
                        ps, lhsT=htb[k][:, m * P:(m + 1) * P],
                        rhs=wv_tiles[k][:, c * FC:(c + 1) * FC],
                        start=(k == 0), stop=(k == NT - 1))
                nc.vector.tensor_add(
                    out=Vsb[m][:, c * 6:(c + 1) * 6, 0:HD],
                    in0=ps.rearrange("p (h d) -> p h d", d=HD),
                    in1=bvb[:, c * FC:(c + 1) * FC].rearrange(
                        "p (h d) -> p h d", d=HD))

        # ---- phase D: product tree G = Wo^T S1..S7 Wp^T ----
        # Every node X@Y consumes (X transposed, Y natural); host pre-orients
        # leaves: chain slots = [S1, S2^T, S3, S4^T, S5, S6^T, S7].
        def mm768(lhsT_tiles, rhs_tiles, out_tiles, psname):
            for m in range(NT):
                for c in range(NFC):
                    ps = pA.tile([P, FC], F32, name=psname, tag="ps_lin")
                    for k in range(NT):
                        nc.tensor.matmul(
                            ps, lhsT=lhsT_tiles[k][:, m * P:(m + 1) * P],
                            rhs=rhs_tiles[k][:, c * FC:(c + 1) * FC],
                            start=(k == 0), stop=(k == NT - 1))
                    if (m * NFC + c) % 2 == 0:
                        nc.scalar.copy(
                            out=out_tiles[m][:, c * FC:(c + 1) * FC], in_=ps)
                    else:
                        nc.vector.tensor_copy(
                            out=out_tiles[m][:, c * FC:(c + 1) * FC], in_=ps)

        tree = {}

        def node(nm, lhsT_tiles, rhs_tiles):
            out = [rp.tile([P, H], BF16, name=f"{nm}{t}", tag="R")
                   for t in range(NT)]
            mm768(lhsT_tiles, rhs_tiles, out, f"ps_{nm}")
            tree[nm] = out
            return out

        def chain_unit(u):
            if u == 0:      # A^T = S1^T @ Wo : lhsT=S1(nat), rhs=Wo(=r0)
                s1 = load_w(chain_t[0], "s1")
                wo = load_w(r0_t, "wo")
                node("AT", s1, wo)
            elif u == 1:    # B = S2 @ S3 : lhsT=S2^T, rhs=S3(nat)
                s2t = load_w(chain_t[1], "s2t")
                s3 = load_w(chain_t[2], "s3")
                node("Bn", s2t, s3)
            elif u == 2:    # C^T = S5^T @ S4^T : lhsT=S5(nat), rhs=S4^T
                s5 = load_w(chain_t[4], "s5")
                s4t = load_w(chain_t[3], "s4t")
                node("CT", s5, s4t)
            elif u == 3:    # D = S6 @ S7 : lhsT=S6^T, rhs=S7(nat)
                s6t = load_w(chain_t[5], "s6t")
                s7 = load_w(chain_t[6], "s7")
                node("Dn", s6t, s7)
            elif u == 4:    # E^T = B^T @ A^T : lhsT=B(nat), rhs=A^T
                node("ET", tree["Bn"], tree["AT"])
            elif u == 5:    # F = C @ D : lhsT=C^T, rhs=D(nat)
                node("Fn", tree["CT"], tree["Dn"])
            elif u == 6:    # G1^T = F^T @ E^T : lhsT=F(nat), rhs=E^T
                node("G1T", tree["Fn"], tree["ET"])
            elif u == 7:    # G = G1 @ Wp^T : lhsT=G1^T, rhs=projT(nat)
                pj = load_w(projT_t, "pj")
                node("Gn", tree["G1T"], pj)

        # ---- phase E: attention per head, chain tree interleaved ----
        ctxu = [per.tile([P, S], BF16, name=f"ctxu{t}", tag=f"ctxu{t}")
                for t in range(NT)]
        dens = []
        unit_at = {0: 0, 1: 1, 2: 2, 3: 3, 5: 4, 6: 5, 8: 6, 9: 7}
        for h in range(NH):
            t_h, p0 = h // 2, (h % 2) * HD
            den = rbp.tile([1, S], F32, name=f"den{h}", tag="den", bufs=4)
            dens.append(den)
            attnT = [ap_.tile([P, S], BF16, name=f"attn{h}_{kt}", tag="attn")
                     for kt in range(NT)]
            for kt in range(NT):
                ps = pS.tile([P, NFC, FC], F32, name="ps_s", tag="ps_s",
                             padded_shape=[P, NFC, 512])
                for c in range(NFC):
                    nc.tensor.matmul(
                        ps[:, c, :],
                        lhsT=KT[t_h][p0:p0 + HD, kt * P:(kt + 1) * P],
                        rhs=QT[t_h][p0:p0 + HD, c * FC:(c + 1) * FC],
                        start=True, stop=True)
                nc.scalar.activation(
                    out=attnT[kt].rearrange("p (c f) -> p c f", f=FC), in_=ps,
                    func=AF.Exp, scale=1.0 / np.sqrt(HD))
            for c in range(NFC):
                ps = pC.tile([HD + 1, FC], F32, name="ps_c", tag="ps_c")
                for kt in range(NT):
                    nc.tensor.matmul(
                        ps, lhsT=Vsb[kt][:, h, :],
                        rhs=attnT[kt][:, c * FC:(c + 1) * FC],
                        start=(kt == 0), stop=(kt == NT - 1))
                nc.vector.tensor_copy(
                    out=ctxu[t_h][p0:p0 + HD, c * FC:(c + 1) * FC],
                    in_=ps[0:HD, :])
                nc.vector.tensor_copy(
                    out=den[0:1, c * FC:(c + 1) * FC],
                    in_=ps[HD:HD + 1, :])
            if h in unit_at:
                chain_unit(unit_at[h])
            # normalize ptile t once both its heads are done
            if h % 2 == 1:
                for c in range(NFC):
                    ps = pA.tile([P, FC], F32, name="ps_rb", tag="ps_lin")
                    nc.tensor.matmul(ps[0:HD, :], lhsT=ones64,
                                     rhs=dens[h - 1][:, c * FC:(c + 1) * FC],
                                     start=True, stop=True)
                    nc.tensor.matmul(ps[HD:P, :], lhsT=ones64,
                                     rhs=dens[h][:, c * FC:(c + 1) * FC],
                                     start=True, stop=True)
                    rb = rbp.tile([P, FC], F32, name="rb", tag="rb")
                    nc.vector.reciprocal_approx_fast(out=rb, in_=ps)
                    nc.vector.tensor_mul(
                        out=ctxu[t_h][:, c * FC:(c + 1) * FC],
                        in0=ctxu[t_h][:, c * FC:(c + 1) * FC], in1=rb)

        # ---- phase G: apply G + b_post + residual -> xT (fp32 + bf16) ----
        G = tree["Gn"]
        xT = [ap_.tile([P, S], F32, name=f"xT{t}", tag="attn") for t in range(NT)]
        xsq = [ap_.tile([P, S], BF16, name=f"xsq{t}", tag=f"xsq{t}", bufs=1)
               for t in range(NT)]
        ones128b = sm.tile([P, 1], BF16)
        nc.vector.memset(ones128b, 1.0)
        ones128f = sm.tile([P, 1], F32)
        nc.vector.memset(ones128f, 1.0)
        ps_mean = pS.tile([1, NFC, FC], F32, name="ps_mean", tag="ps_s",
                          padded_shape=[1, NFC, 512])
        ps_sq = pS.tile([1, NFC, FC], F32, name="ps_sq", tag="ps_s",
                        padded_shape=[1, NFC, 512])
        for m in range(NT):
            for c in range(NFC):
                ps = pA.tile([P, FC], F32, name="ps_app", tag="ps_lin")
                for k in range(NT):
                    nc.tensor.matmul(
                        ps, lhsT=G[k][:, m * P:(m + 1) * P],
                        rhs=ctxu[k][:, c * FC:(c + 1) * FC],
                        start=(k == 0), stop=(k == NT - 1))
                sl = slice(c * FC, (c + 1) * FC)
                nc.vector.scalar_tensor_tensor(
                    out=xT[m][:, sl], in0=ps,
                    scalar=bias_sb[:, m, 2:3], in1=ht[m][:, sl],
                    op0=ALU.add, op1=ALU.add)
                nc.gpsimd.tensor_mul(out=xsq[m][:, sl], in0=xT[m][:, sl],
                                     in1=xT[m][:, sl])
                # accumulate layernorm stats as soon as tile m is ready
                nc.tensor.matmul(ps_mean[:, c, :], lhsT=ones128f,
                                 rhs=xT[m][:, sl],
                                 start=(m == 0), stop=(m == NT - 1))
                nc.tensor.matmul(ps_sq[:, c, :], lhsT=ones128b,
                                 rhs=xsq[m][:, sl],
                                 start=(m == 0), stop=(m == NT - 1))

        # ---- phase I: layernorm rows ----
        # rows: m = mean/H ; v = sumsq/H - m^2 ; rstd = 1/sqrt(v + eps)
        m_row = sm.tile([1, S], F32)
        nc.vector.tensor_scalar_mul(
            out=m_row.rearrange("p (c f) -> p c f", f=FC), in0=ps_mean,
            scalar1=1.0 / H)
        msq_row = sm.tile([1, S], F32)
        nc.vector.tensor_mul(out=msq_row, in0=m_row, in1=m_row)
        v_row = sm.tile([1, S], F32)
        nc.vector.scalar_tensor_tensor(
            out=v_row.rearrange("p (c f) -> p c f", f=FC), in0=ps_sq,
            scalar=1.0 / H, in1=msq_row.rearrange("p (c f) -> p c f", f=FC),
            op0=ALU.mult, op1=ALU.subtract)
        sd_row = sm.tile([1, S], F32)
        nc.scalar.activation(out=sd_row, in_=v_row, func=AF.Sqrt,
                             bias=eps_t[0:1, :], scale=1.0)
        rstd_row = sm.tile([1, S], F32)
        nc.vector.reciprocal_approx_fast(out=rstd_row, in_=sd_row)
        # broadcast mean and rstd to all partitions (K=1 ones matmul)
        ps_mb = pS.tile([P, NFC, FC], F32, name="ps_mb", tag="ps_s",
                        padded_shape=[P, NFC, 512])
        ps_rb = pS.tile([P, NFC, FC], F32, name="ps_rb2", tag="ps_s",
                        padded_shape=[P, NFC, 512])
        for c in range(NFC):
            sl = slice(c * FC, (c + 1) * FC)
            nc.tensor.matmul(ps_mb[:, c, :], lhsT=ones1, rhs=m_row[:, sl],
                             start=True, stop=True)
            nc.tensor.matmul(ps_rb[:, c, :], lhsT=ones1, rhs=rstd_row[:, sl],
                             start=True, stop=True)
        if _dbg:
            for t in range(NT):
                nc.sync.dma_start(out=d_dbg[t], in_=xT[t])
            nc.sync.dma_start(out=d_dbgrow[0:1], in_=m_row)
            nc.sync.dma_start(out=d_dbgrow[1:2], in_=v_row)
            nc.sync.dma_start(out=d_dbgrow[2:3], in_=rstd_row)
        # apply: out^T = (xT - meanb) * (rstdb * g[h]) + b[h]
        for t in range(NT):
            rg = bc.tile([P, S], F32, name="rg", tag="bcast")
            nc.vector.tensor_scalar_mul(
                out=rg.rearrange("p (c f) -> p c f", f=FC), in0=ps_rb,
                scalar1=bias_sb[:, t, 3:4])
            xo = bc.tile([P, S], F32, name="xo", tag="bcast")
            nc.vector.tensor_tensor(
                out=xo.rearrange("p (c f) -> p c f", f=FC),
                in0=xT[t].rearrange("p (c f) -> p c f", f=FC),
                in1=ps_mb, op=ALU.subtract)
            nc.vector.tensor_mul(out=xo, in0=xo, in1=rg)
            nc.vector.tensor_scalar_add(out=xo, in0=xo,
                                        scalar1=bias_sb[:, t, 4:5])
            nc.sync.dma_start(out=out_t[t], in_=xo)

    nc.finalize()
    return nc


def _prep_inputs(hidden_states, language_ids, Wq_lang, bq_lang, Wk_lang, bk_lang,
                 in_proj_w, in_proj_b, out_proj_w, out_proj_b, align,
                 proj_w, proj_b, ln_g, ln_b):
    f = np.float32
    hs = np.asarray(hidden_states, f)
    lang = np.asarray(language_ids).astype(np.int64)
    Wq_lang = np.asarray(Wq_lang, f)
    bq_lang = np.asarray(bq_lang, f)
    Wk_lang = np.asarray(Wk_lang, f)
    bk_lang = np.asarray(bk_lang, f)
    in_proj_w = np.asarray(in_proj_w, f)
    in_proj_b = np.asarray(in_proj_b, f)
    out_proj_w = np.asarray(out_proj_w, f)
    out_proj_b = np.asarray(out_proj_b, f)
    align = np.asarray(align, f)
    proj_w = np.asarray(proj_w, f)
    proj_b = np.asarray(proj_b, f)
    ln_g = np.asarray(ln_g, f)
    ln_b = np.asarray(ln_b, f)

    wq, wk, wv = in_proj_w[:H], in_proj_w[H:2 * H], in_proj_w[2 * H:]
    bq_, bk_, bv_ = in_proj_b[:H], in_proj_b[H:2 * H], in_proj_b[2 * H:]
    projT = np.ascontiguousarray(proj_w.T)
    wvT = np.ascontiguousarray(wv.T)
    identity = np.eye(H, dtype=f)

    langs_present = sorted(set(lang.tolist()))
    wcq, wck, bcq, bck = {}, {}, {}, {}
    for l in langs_present:
        wcq[l] = np.ascontiguousarray((wq @ Wq_lang[l]).T).astype(BF)
        wck[l] = np.ascontiguousarray((wk @ Wk_lang[l]).T).astype(BF)
        bcq[l] = wq @ bq_lang[l] + bq_
        bck[l] = wk @ bk_lang[l] + bk_

    in_maps = []
    for i in range(B):
        li = int(lang[i])
        factors = []
        v = out_proj_b.copy()
        for j in range(B):
            if j == i:
                continue
            if int(lang[j]) == li:
                factors.append(identity)
            else:
                Aij = align[li, int(lang[j])]
                factors.append(Aij)
                v = v @ Aij
        b_post = v @ projT + proj_b
        # tree leaf orientation: transpose slots 1, 3, 5 (S2, S4, S6)
        slots = [factors[0], factors[1].T, factors[2], factors[3].T,
                 factors[4], factors[5].T, factors[6]]
        bias_pack = np.stack(
            [bcq[li].reshape(NT, P), bck[li].reshape(NT, P),
             b_post.reshape(NT, P), ln_g.reshape(NT, P),
             ln_b.reshape(NT, P)], axis=-1)
        rows_pack = bv_.reshape(1, S)
        in_maps.append({
            "ht": np.ascontiguousarray(hs[i].T),
            "htb": np.ascontiguousarray(hs[i].T).astype(BF),
            "wcq": wcq[li],
            "wck": wck[li],
            "wv": wvT.astype(BF),
            "r0": out_proj_w.astype(BF),
            "chain": np.ascontiguousarray(np.stack(slots, axis=0)).astype(BF),
            "projT": projT.astype(BF),
            "bias_pack": np.ascontiguousarray(bias_pack.astype(f)),
            "rows_pack": np.ascontiguousarray(rows_pack.astype(f)),
        })
    return in_maps


def kernel(hidden_states, language_ids, attention_mask, Wq_lang, bq_lang,
           Wk_lang, bk_lang, in_proj_w, in_proj_b, out_proj_w, out_proj_b,
           align, proj_w, proj_b, ln_g, ln_b):
    global _CACHED_NC, LAST_RESULTS
    if _CACHED_NC is None:
        _CACHED_NC = _build_program()
    in_maps = _prep_inputs(hidden_states, language_ids, Wq_lang, bq_lang,
                           Wk_lang, bk_lang, in_proj_w, in_proj_b,
                           out_proj_w, out_proj_b, align, proj_w, proj_b,
                           ln_g, ln_b)
    res = run_bass_kernel_spmd(_CACHED_NC, in_maps, core_ids=list(range(B)))
    LAST_RESULTS = res
    return np.stack([np.ascontiguousarray(res.results[i]["out"].T)
                     for i in range(B)], axis=0)
